# revision 29
# baseline (speedup 1.0000x reference)
"""Trainium2 Bass kernel for nn_DigitConvolutionalModel.

Model: x(B,784) -> reshape 28x28 -> 3x3 valid cross-correlation (kernel is an
input) -> flatten 676 -> Linear(676,128)+ReLU -> Linear(128,10).

Strategy:
  * Fold the 3x3 conv into the first linear layer on the host: the conv is a
    linear map, so h = relu(x @ W1eff.T + b1) with W1eff (128, 784) built by
    scattering conv_w-weighted copies of w1 onto the 28x28 grid. The device
    kernel is then a plain 2-layer MLP over 784 features.
  * Pure data parallelism: batch 65536 split as 8192 rows per NeuronCore,
    weights replicated.
  * Activations are shipped feature-major (x.T per shard) so the contraction
    dim lands on SBUF partitions with fully contiguous DMA; the kernel
    computes logits^T = w2 @ relu(W1eff @ x^T + b1) + b2 and the host
    transposes the gathered (10, B) result back.
  * x is DMAed in 6.4 MB blocks (4 per core) for near-peak HBM efficiency.
"""

from contextlib import ExitStack

import numpy as np

B = 65536
H = W = 28
K = 3
CH = CW = 26
FEAT = H * W          # 784
HID = 128
OUT = 10
NCORES = 8
BC = B // NCORES      # 8192 rows per core

KC = 112              # contraction-chunk partition size
KCH = 7               # chunks: 7 * 112 = 784
NT = 512              # batch rows per compute tile (one PSUM bank at fp32)
XB = 1024             # batch rows per DMA block

# "f32"   : exact fp32 matmuls (slowest PE: 4 cyc/row + 4B self-load)
# "f32r"  : fp32r operands everywhere (1 cyc/row, still 4B self-load)
# "bf16"  : bf16 operands everywhere, x shipped bf16 (half DMA, FWL loads)
# "f16"   : float16 operands everywhere, x shipped fp16 (half DMA, FWL loads)
VARIANT = "f16"

_NC_CACHE = {}


def _dtypes(variant):
    import concourse.mybir as mybir

    f32 = mybir.dt.float32
    if variant == "f32":
        return f32, f32
    if variant == "f32r":
        return mybir.dt.float32r, mybir.dt.float32r
    if variant == "bf16":
        return mybir.dt.bfloat16, mybir.dt.bfloat16
    if variant == "f16":
        return mybir.dt.float16, mybir.dt.float16
    raise ValueError(variant)


def _build_nc(bc, variant):
    from concourse import bacc
    import concourse.mybir as mybir
    import concourse.tile as tile

    f32 = mybir.dt.float32
    wdt, xdt = _dtypes(variant)
    # descending block sizes: big DMAs early (efficiency), small final
    # block so the post-DMA compute tail is short
    if bc == 8192:
        blocks = [2048, 2048, 2048, 1024, 512, 512]
    else:
        blocks = [min(XB, bc - o) for o in range(0, bc, XB)]
    assert sum(blocks) == bc and all(b % NT == 0 for b in blocks)

    nc = bacc.Bacc(
        "TRN2",
        target_bir_lowering=False,
        debug=False,
        enable_asserts=False,
        num_devices=NCORES,
    )
    xT = nc.dram_tensor("xT", [KC, KCH, bc], xdt, kind="ExternalInput").ap()
    w1t = nc.dram_tensor("w1t", [KC, KCH, HID], wdt, kind="ExternalInput").ap()
    b1 = nc.dram_tensor("b1", [HID, 1], f32, kind="ExternalInput").ap()
    w2t = nc.dram_tensor("w2t", [HID, OUT], wdt, kind="ExternalInput").ap()
    b2 = nc.dram_tensor("b2", [OUT, 1], f32, kind="ExternalInput").ap()
    outT = nc.dram_tensor("outT", [OUT, bc], f32, kind="ExternalOutput").ap()

    with ExitStack() as ctx:
        tc = ctx.enter_context(tile.TileContext(nc))
        wpool = ctx.enter_context(tc.tile_pool(name="w", bufs=1))
        xpool = ctx.enter_context(tc.tile_pool(name="x", bufs=4))
        hpool = ctx.enter_context(tc.tile_pool(name="h", bufs=3))
        opool = ctx.enter_context(tc.tile_pool(name="o", bufs=2))
        p1pool = ctx.enter_context(tc.tile_pool(name="p1", bufs=6, space="PSUM"))
        p2pool = ctx.enter_context(tc.tile_pool(name="p2", bufs=2, space="PSUM"))

        # weights go through the ACT HWDGE ring so the sync ring's FIFO is
        # free for the (big) x-block loads from instruction zero
        w1s = wpool.tile([KC, KCH, HID], wdt)
        nc.scalar.dma_start(w1s[:], w1t[:])
        b1s = wpool.tile([HID, 1], f32)
        nc.scalar.dma_start(b1s[:], b1[:])
        w2s = wpool.tile([HID, OUT], wdt)
        nc.scalar.dma_start(w2s[:], w2t[:])
        b2s = wpool.tile([OUT, 1], f32)
        nc.scalar.dma_start(b2s[:], b2[:])

        add = mybir.AluOpType.add
        mx = mybir.AluOpType.max

        off = 0
        for blk, xb in enumerate(blocks):
            tpb = xb // NT
            xs = xpool.tile([KC, KCH, xb], xdt, tag="xs", name=f"xs_{blk}")
            nc.sync.dma_start(xs[:], xT[:, :, off : off + xb])
            os_ = opool.tile([OUT, xb], f32, tag="os", name=f"os_{blk}")
            # chunk-outer order: consecutive matmuls share the stationary
            # operand, so weight (re)loads pipeline behind the streams
            p1s = [
                p1pool.tile([HID, NT], f32, tag="p1", name=f"p1_{blk}_{t}")
                for t in range(tpb)
            ]
            for c in range(KCH):
                for t in range(tpb):
                    nc.tensor.matmul(
                        p1s[t][:],
                        w1s[:, c, :],
                        xs[:, c, t * NT : (t + 1) * NT],
                        start=(c == 0),
                        stop=(c == KCH - 1),
                    )
            for t in range(tpb):
                sl = slice(t * NT, (t + 1) * NT)
                # epilogue entirely on the (otherwise idle) vector engine
                hs = hpool.tile([HID, NT], xdt, tag="hs", name=f"hs_{blk}_{t}")
                nc.vector.tensor_scalar(hs[:], p1s[t][:], b1s[:], 0.0, add, mx)
                p2 = p2pool.tile([OUT, NT], f32, tag="p2", name=f"p2_{blk}_{t}")
                nc.tensor.matmul(p2[:], w2s[:], hs[:], start=True, stop=True)
                nc.vector.tensor_scalar_add(os_[:, sl], p2[:], b2s[:])
            nc.scalar.dma_start(outT[:, off : off + xb], os_[:])
            off += xb

    nc.compile()
    return nc


def get_nc(bc=BC, variant=VARIANT):
    key = (bc, variant)
    if key not in _NC_CACHE:
        _NC_CACHE[key] = _build_nc(bc, variant)
    return _NC_CACHE[key]


def _np_wdt(variant):
    if variant == "bf16":
        import ml_dtypes

        return ml_dtypes.bfloat16
    if variant == "f16":
        return np.float16
    return np.float32


def _host_prep(x, conv_w, w1, b1, w2, b2, variant):
    """Fold conv into layer-1 weights and lay out per-core device inputs."""
    x = np.asarray(x, dtype=np.float32)
    conv_w = np.asarray(conv_w, dtype=np.float32)
    w1 = np.asarray(w1, dtype=np.float32)
    b1 = np.asarray(b1, dtype=np.float32)
    w2 = np.asarray(w2, dtype=np.float32)
    b2 = np.asarray(b2, dtype=np.float32)

    w1_img = w1.reshape(HID, CH, CW)
    w1eff = np.zeros((HID, H, W), dtype=np.float32)
    for di in range(K):
        for dj in range(K):
            w1eff[:, di : di + CH, dj : dj + CW] += conv_w[di, dj] * w1_img
    w1eff = w1eff.reshape(HID, FEAT)

    wnp = _np_wdt(variant)
    # [784,128] -> [7,112,128] -> [112,7,128] so chunk c partition p holds
    # feature c*112+p
    w1t_host = np.ascontiguousarray(
        w1eff.T.reshape(KCH, KC, HID).transpose(1, 0, 2)
    ).astype(wnp)
    b1_host = np.ascontiguousarray(b1.reshape(HID, 1))
    w2t_host = np.ascontiguousarray(w2.T).astype(wnp)
    b2_host = np.ascontiguousarray(b2.reshape(OUT, 1))

    in_maps = []
    for c in range(NCORES):
        shard = x[c * BC : (c + 1) * BC].T  # [784, BC] view
        # [784, BC] -> [112, 7, BC]: chunk dim in the middle
        shard = np.ascontiguousarray(
            shard.reshape(KCH, KC, BC).transpose(1, 0, 2)
        ).astype(wnp, copy=False)
        in_maps.append(
            {
                "xT": shard,
                "w1t": w1t_host,
                "b1": b1_host,
                "w2t": w2t_host,
                "b2": b2_host,
            }
        )
    return in_maps


def run(x, conv_w, w1, b1, w2, b2, trace=False, variant=VARIANT):
    from concourse.bass_utils import run_bass_kernel_spmd

    in_maps = _host_prep(x, conv_w, w1, b1, w2, b2, variant)
    nc = get_nc(BC, variant)
    res = run_bass_kernel_spmd(nc, in_maps, list(range(NCORES)), trace=trace)
    outT = np.concatenate([r["outT"] for r in res.results], axis=1)  # [10, B]
    return np.ascontiguousarray(outT.T), res


def kernel(x, conv_w, w1, b1, w2, b2):
    out, _ = run(x, conv_w, w1, b1, w2, b2)
    return out
